# revision 39
# baseline (speedup 1.0000x reference)
"""GCN decoder kernel for Trainium2, 8-core data-parallel over batch.

Key idea: the mask is random 0/1, so only K~=1024 of 2048 nodes are active
per sample, and all masked rows/cols of the output are exactly zero (pair
mask). The computation is permutation-equivariant, so the host packs the
active nodes first (padded to KN = 128*ceil(Kmax/128)) and the device runs
the whole pipeline on [KN, KN] ~= 1/4 of the work. The host scatters the
[K, K] result back into a zero [N, N] matrix.

Host precomputes Xn = X/||X|| (shipped as XnT fp16) so the device does:
  S   = XnT^T XnT;  sig = sigmoid(S) + diag(1)      (f16, ACT)
  deg = rowsum(sig) - 0.5*(KN-K);  d = rsqrt(max(deg, 1e-6))   (DVE,
        rsqrt via bit-hack + 3 Newton steps -- no ACT table switch)
  aggT = d_i * (Y^T @ sig),  Y = d*X (f16)          == (A_norm @ X)^T
  HfT = relu(W1^T aggT + b1);  PT = W2^T HfT + b2   (DVE bias/relu)
  out = sigmoid(PT^T PT)  (f16)  -> host slices [K, K] and scatters.

Padded rows have X=0 -> Xn=0 -> S cols exact 0 -> sigmoid 0.5, corrected in
deg by the host-provided 0.5*(KN-K); padded rows of Y are 0 so they never
contribute; padded output rows/cols are discarded on host.
"""

from contextlib import ExitStack

import numpy as np

import bass_rust as _bass_rust
import concourse.bass as bass
import concourse.mybir as mybir
import concourse.tile as tile
from concourse.bass_utils import run_bass_kernel_spmd
from concourse.masks import make_identity

F32 = mybir.dt.float32
F16 = mybir.dt.float16
I32 = mybir.dt.int32
AF = mybir.ActivationFunctionType
OP = mybir.AluOpType

B = 8
N = 2048
D = 256
H = 256
P = 128
MAGIC1 = 0x5F3759DF + 1  # fp32 rsqrt bit-hack constant (+1 for the xor trick)


def _install_drain_split(max_waits: int = 1):
    """This walrus build accepts at most ONE sync-wait per instruction.
    (a) split the Tile kernel-tail drain into single-wait drains;
    (b) hoist extra waits from any lowered instruction onto standalone
    EventSemaphore instructions on the same engine."""
    from concourse.vector_clock import ScopedClock

    if getattr(tile.TileContext, "_drain_split_installed", False):
        return

    def _drain_and_barrier(self, tick_clock, wait_clock):
        drain_inst = self.nc.sync.drain()
        wait_clock.add_sem_waits(
            drain_inst.ins, ScopedClock({None: tick_clock.global_clock})
        )
        si = drain_inst.ins.sync_info
        waits = list(si.on_wait) if si is not None and si.on_wait else []
        if len(waits) > max_waits:
            drain_inst.ins.sync_info = _bass_rust.SyncInfo(
                on_wait=waits[:max_waits],
                on_update=list(si.on_update) if si.on_update else [],
            )
            rest = waits[max_waits:]
            for i in range(0, len(rest), max_waits):
                extra = self.nc.sync.drain()
                extra.ins.sync_info = _bass_rust.SyncInfo(
                    on_wait=rest[i : i + max_waits], on_update=[]
                )
        self.nc.all_engine_barrier()
        assert self.sems is not None
        popped = self.nc._tile_sem_poison_stack.pop()
        assert popped is self._sem_poison
        self.nc.clear_and_free_semaphores(list(self.sems.allocated().values()))
        self.nc.all_engine_barrier()

    tile.TileContext._drain_and_barrier = _drain_and_barrier

    orig_add = tile.TileContext._add_instruction
    counter = [0]

    def _add_instruction(self, inst):
        si = inst.sync_info
        if si is not None and si.on_wait and len(si.on_wait) > max_waits:
            waits = list(si.on_wait)
            keep = waits[-max_waits:]
            for w in waits[: -max_waits]:
                counter[0] += 1
                ev = mybir.InstEventSemaphore(
                    name=f"{inst.name}-xw{counter[0]}", ins=[], outs=[]
                )
                ev.engine = inst.engine
                ev.sync_info = _bass_rust.SyncInfo(on_wait=[w], on_update=[])
                orig_add(self, ev)
            inst.sync_info = _bass_rust.SyncInfo(
                on_wait=keep, on_update=list(si.on_update) if si.on_update else []
            )
        orig_add(self, inst)

    tile.TileContext._add_instruction = _add_instruction
    tile.TileContext._drain_split_installed = True


def build_nc(kb: int):
    _install_drain_split()
    KN = P * kb
    KNP = max(KN, 1024)  # psum tile width (>= 1024 for the agg 2-half layout)
    chunks = [(c, min(512, KN - c)) for c in range(0, KN, 512)]

    nc = bass.Bass("TRN2", target_bir_lowering=False, debug=False, num_devices=B)

    xnt_d = nc.dram_tensor("xnt", [P, 2, KN], F16, kind="ExternalInput").ap()
    x16_d = nc.dram_tensor("x16", [KN, D], F16, kind="ExternalInput").ap()
    w1_d = nc.dram_tensor("w1", [D, H], F16, kind="ExternalInput").ap()
    w2_d = nc.dram_tensor("w2", [H, H], F16, kind="ExternalInput").ap()
    b1_d = nc.dram_tensor("b1t", [P, H // P], F32, kind="ExternalInput").ap()
    b2_d = nc.dram_tensor("b2t", [P, H // P], F32, kind="ExternalInput").ap()
    cv_d = nc.dram_tensor("cvec", [P, 1], F32, kind="ExternalInput").ap()
    on_d = nc.dram_tensor("ones16", [1, P], F16, kind="ExternalInput").ap()
    out_d = nc.dram_tensor("out", [KN, KN], F16, kind="ExternalOutput").ap()

    with tile.TileContext(nc) as tc:
        with ExitStack() as top:
            const = top.enter_context(tc.tile_pool(name="const", bufs=1))
            psum = top.enter_context(tc.tile_pool(name="psum", bufs=2, space="PSUM"))
            psd = top.enter_context(tc.tile_pool(name="psd", bufs=1, space="PSUM"))
            outp = top.enter_context(tc.tile_pool(name="outp", bufs=3))

            # ---- input DMAs first (xnt gates phase 2): one tile per
            # (k, 512-col chunk) so the first S matmuls start as soon as
            # the first chunk lands; spread across the three DMA queues ----
            h0 = (KN // 2) // P * P
            xnt = [[None] * len(chunks) for _ in range(2)]
            dma_order = [nc.sync, nc.gpsimd, nc.sync, nc.scalar, nc.scalar, nc.scalar]
            di = 0
            for k in range(2):
                for ci, (c0, cw) in enumerate(chunks):
                    t = const.tile([P, cw], F16, tag=f"xnt{k}_{ci}")
                    dma_order[di % len(dma_order)].dma_start(
                        t[:], xnt_d[:, k, c0 : c0 + cw]
                    )
                    di += 1
                    xnt[k][ci] = t
            x16 = const.tile([P, kb, D], F16, tag="x16")
            nc.gpsimd.dma_start(x16[:], x16_d.rearrange("(b p) d -> p b d", p=P))
            w1 = const.tile([P, 2, H], F16, tag="w1")
            nc.gpsimd.dma_start(w1[:], w1_d.rearrange("(c p) h -> p c h", p=P))
            w2 = const.tile([P, 2, H], F16, tag="w2")
            nc.gpsimd.dma_start(w2[:], w2_d.rearrange("(c p) h -> p c h", p=P))
            b1v = const.tile([P, 2], F32, tag="b1v")
            nc.sync.dma_start(b1v[:], b1_d[:])
            b2v = const.tile([P, 2], F32, tag="b2v")
            nc.sync.dma_start(b2v[:], b2_d[:])
            cvec = const.tile([P, 1], F32, tag="cvec")
            nc.sync.dma_start(cvec[:], cv_d[:])
            ones1 = const.tile([1, P], F16, tag="ones1")
            nc.sync.dma_start(ones1[:], on_d[:])

            # warm up the ACT sigmoid table set right away
            warm = const.tile([1, 2], F32, tag="warm")
            nc.vector.memset(warm[0:1, 0:1], 0.0)
            nc.scalar.activation(warm[0:1, 1:2], warm[0:1, 0:1], AF.Sigmoid)

            eye = const.tile([P, P], F32, tag="eye")
            make_identity(nc, eye[:])

            sig = const.tile([P, kb, KN], F16, tag="sig")
            y16 = const.tile([P, kb, D], F16, tag="y16")
            aggt = const.tile([P, 2, KN], F16, tag="aggt")
            hft = const.tile([P, 2, KN], F16, tag="hft")
            ptt = const.tile([P, 2, KN], F16, tag="ptt")
            dbc = const.tile([P, KN], F16, tag="dbc")
            drow = const.tile([1, KN], F16, tag="drow")
            d16 = const.tile([16, P], F16, tag="d16")
            dgv = const.tile([P, kb], F32, tag="dgv")
            dxh = const.tile([P, kb], F32, tag="dxh")
            dsy = const.tile([P, kb], F32, tag="dsy")
            dnt = const.tile([P, kb], F32, tag="dnt")

            # ---- phase 2: S = Xn Xn^T, sigmoid (f16), diag, rowsum ----
            # deg rowsum: early blocks on the Vector reduce, late blocks via
            # the ACT accumulator (read right after the sigmoid) — balances
            # the two engines; diag +1 is folded into the host cvec.
            NV = min(kb, (kb + 1) // 2)
            for jb in range(kb):
                jsl = slice(jb * P, (jb + 1) * P)
                ps = psum.tile([P, KNP], F32, tag="big")
                jc, joff = (jb * P) // 512, (jb * P) % 512
                for k in range(2):
                    for ci, (c0, cw) in enumerate(chunks):
                        nc.tensor.matmul(
                            ps[:, c0 : c0 + cw],
                            xnt[k][jc][:, joff : joff + P],
                            xnt[k][ci][:, 0:cw],
                            start=(k == 0),
                            stop=(k == 1),
                        )
                if jb < NV:
                    nc.scalar.activation(sig[:, jb, :], ps[:, 0:KN], AF.Sigmoid)
                    nc.vector.tensor_reduce(
                        out=dgv[:, jb : jb + 1],
                        in_=sig[:, jb, :],
                        axis=mybir.AxisListType.X,
                        op=OP.add,
                    )
                else:
                    nc.scalar.activation(
                        sig[:, jb, :],
                        ps[:, 0:KN],
                        AF.Sigmoid,
                        accum_out=dgv[:, jb : jb + 1],
                    )
                # add identity on the diagonal block (self loops; padded
                # rows' +1 only touches discarded columns)
                nc.vector.scalar_tensor_tensor(
                    out=sig[:, jb, jsl],
                    in0=eye[:],
                    scalar=1.0,
                    in1=sig[:, jb, jsl],
                    op0=OP.mult,
                    op1=OP.add,
                )

            # ---- phase 3: d = rsqrt(max(deg - cvec, 1e-6)), Y, dbc ----
            # Keep-warm matmuls: the PE would otherwise idle > 3.4us across
            # phase 3 and HAM would re-throttle it to 1.2 GHz for phase 4's
            # first chunk. Reading sig[jb] staggers them behind the last
            # sigmoids so the warmth spans the whole gap; unread results.
            for jb in range(max(0, kb - 3), kb):
                for _ in range(3):
                    pwm = psd.tile([P, 1024], F32, tag="psd")
                    nc.tensor.matmul(
                        pwm[:, 0:512],
                        sig[:, jb, 0:P],
                        sig[:, jb, 0:512],
                        start=True,
                        stop=True,
                    )
            nc.vector.tensor_scalar(
                out=dgv[:],
                in0=dgv[:],
                scalar1=cvec[:, 0:1],
                scalar2=1e-6,
                op0=OP.subtract,
                op1=OP.max,
            )
            nc.vector.tensor_scalar_mul(dxh[:], dgv[:], -0.5)  # -x/2
            # y0 bits = MAGIC - (i >> 1)  ==  ((i >> 1) ^ -1) + (MAGIC + 1)
            nc.vector.tensor_scalar(
                out=dsy[:].bitcast(I32),
                in0=dgv[:].bitcast(I32),
                scalar1=1,
                scalar2=-1,
                op0=OP.logical_shift_right,
                op1=OP.bitwise_xor,
            )
            nc.vector.tensor_scalar_add(
                dsy[:].bitcast(I32), dsy[:].bitcast(I32), MAGIC1
            )
            for _ in range(2):  # Newton: y *= 1.5 - 0.5*x*y^2 (err ~5e-6)
                nc.vector.tensor_tensor(dnt[:], dsy[:], dsy[:], op=OP.mult)
                nc.vector.tensor_tensor(dnt[:], dnt[:], dxh[:], op=OP.mult)
                nc.vector.tensor_scalar_add(dnt[:], dnt[:], 1.5)
                nc.vector.tensor_tensor(dsy[:], dsy[:], dnt[:], op=OP.mult)
            # dbc[p, i] = d_i for all p: transpose d, SBUF-SBUF DMA to one
            # row, rank-1 broadcasts. Kick the transpose + DMA chain first;
            # the rank-1 matmuls are emitted after phase 4's first chunk so
            # the PE queue is not head-of-line blocked on the DMA latency.
            ptd = psd.tile([P, 1024], F32, tag="psd")
            nc.tensor.transpose(ptd[0:kb, 0:P], dsy[:], eye[:])
            nc.vector.tensor_copy(out=d16[0:kb, :], in_=ptd[0:kb, 0:P])
            nc.sync.dma_start(
                drow[0:1].rearrange("p (o q) -> p o q", o=kb), d16[0:kb, :]
            )
            # Y = d * X
            for jb in range(kb):
                nc.vector.tensor_scalar_mul(
                    y16[:, jb, :], x16[:, jb, :], dsy[:, jb : jb + 1]
                )

            # ---- phase 4: aggT = dbc * (Y^T @ sig) ----
            for ci, (c0, cw) in enumerate(chunks):
                ps = psum.tile([P, KNP], F32, tag="big")
                for jb in range(kb):
                    nc.tensor.matmul(
                        ps[:, 0:cw],
                        y16[:, jb, 0:P],
                        sig[:, jb, c0 : c0 + cw],
                        start=(jb == 0),
                        stop=(jb == kb - 1),
                    )
                    nc.tensor.matmul(
                        ps[:, 512 : 512 + cw],
                        y16[:, jb, P : 2 * P],
                        sig[:, jb, c0 : c0 + cw],
                        start=(jb == 0),
                        stop=(jb == kb - 1),
                    )
                if ci == 0:
                    # rank-1 d broadcasts, behind chunk 0's matmuls (the
                    # drow DMA has landed by now); dedicated 2-bank pool so
                    # the main psum rotation is not disturbed
                    for g0 in range(0, KN, 1024):
                        gw = min(1024, KN - g0)
                        pbd = psd.tile([P, 1024], F32, tag="psd")
                        for b0 in range(0, gw, 512):
                            bw = min(512, gw - b0)
                            nc.tensor.matmul(
                                pbd[:, b0 : b0 + bw],
                                ones1[:],
                                drow[0:1, g0 + b0 : g0 + b0 + bw],
                                start=True,
                                stop=True,
                            )
                        nc.vector.tensor_copy(
                            out=dbc[:, g0 : g0 + gw], in_=pbd[:, 0:gw]
                        )
                nc.vector.tensor_tensor(
                    aggt[:, 0, c0 : c0 + cw],
                    ps[:, 0:cw],
                    dbc[:, c0 : c0 + cw],
                    op=OP.mult,
                )
                nc.vector.tensor_tensor(
                    aggt[:, 1, c0 : c0 + cw],
                    ps[:, 512 : 512 + cw],
                    dbc[:, c0 : c0 + cw],
                    op=OP.mult,
                )

            # ---- phase 5: HfT = relu(W1^T aggT + b1), PT = W2^T HfT + b2 ----
            # k-outer with both psum buffers live, so the k=0 matmuls of the
            # second GEMM can run while the first half's drains finish
            pw = [
                psum.tile([P, KNP], F32, tag="big", name=f"pw1_{hb}")
                for hb in range(2)
            ]
            for k in range(2):
                for hb in range(2):
                    hsl = slice(hb * P, (hb + 1) * P)
                    for c0, cw in chunks:
                        nc.tensor.matmul(
                            pw[hb][:, c0 : c0 + cw],
                            w1[:, k, hsl],
                            aggt[:, k, c0 : c0 + cw],
                            start=(k == 0),
                            stop=(k == 1),
                        )
            # hb0 (which gates the next GEMM's k=0 matmuls) drains on ACT,
            # hb1 on Vector, in parallel
            nc.scalar.activation(
                hft[:, 0, :], pw[0][:, 0:KN], AF.Relu, bias=b1v[:, 0:1]
            )
            nc.vector.tensor_scalar(
                out=hft[:, 1, :],
                in0=pw[1][:, 0:KN],
                scalar1=b1v[:, 1:2],
                scalar2=0.0,
                op0=OP.add,
                op1=OP.max,
            )
            pv = [
                psum.tile([P, KNP], F32, tag="big", name=f"pw2_{hb}")
                for hb in range(2)
            ]
            for k in range(2):
                for hb in range(2):
                    hsl = slice(hb * P, (hb + 1) * P)
                    for c0, cw in chunks:
                        nc.tensor.matmul(
                            pv[hb][:, c0 : c0 + cw],
                            w2[:, k, hsl],
                            hft[:, k, c0 : c0 + cw],
                            start=(k == 0),
                            stop=(k == 1),
                        )
            nc.scalar.activation(
                ptt[:, 0, :], pv[0][:, 0:KN], AF.Identity, bias=b2v[:, 0:1]
            )
            nc.vector.tensor_scalar_add(ptt[:, 1, :], pv[1][:, 0:KN], b2v[:, 1:2])

            # ---- phase 6: out = sigmoid(PT^T PT) (f16), DMA out ----
            dma_engs = [nc.sync, nc.gpsimd]
            for jb in range(kb):
                jsl = slice(jb * P, (jb + 1) * P)
                ps = psum.tile([P, KNP], F32, tag="big")
                for k in range(2):
                    for c0, cw in chunks:
                        nc.tensor.matmul(
                            ps[:, c0 : c0 + cw],
                            ptt[:, k, jsl],
                            ptt[:, k, c0 : c0 + cw],
                            start=(k == 0),
                            stop=(k == 1),
                        )
                osb = outp.tile([P, KN], F16, tag="osb")
                nc.scalar.activation(osb[:], ps[:, 0:KN], AF.Sigmoid)
                dma_engs[jb % 2].dma_start(out_d[jsl, :], osb[:])

    return nc


_NC_CACHE: dict[int, object] = {}


def _get_nc(kb: int):
    if kb not in _NC_CACHE:
        _NC_CACHE[kb] = build_nc(kb)
    return _NC_CACHE[kb]


def _plan(mask):
    """Active indices per sample and the shared padded size KN."""
    idxs = [np.nonzero(np.asarray(mask[b]) != 0)[0] for b in range(mask.shape[0])]
    kmax = max((len(i) for i in idxs), default=1)
    kb = max(1, -(-kmax // P))
    return idxs, kb


def make_in_maps(X, mask, W1, b1, W2, b2, idxs, kb):
    KN = P * kb
    X = np.asarray(X, dtype=np.float32)
    W1 = np.asarray(W1, dtype=np.float32)
    b1 = np.asarray(b1, dtype=np.float32)
    W2 = np.asarray(W2, dtype=np.float32)
    b2 = np.asarray(b2, dtype=np.float32)

    b1t = np.ascontiguousarray(b1.reshape(H // P, P).T)
    b2t = np.ascontiguousarray(b2.reshape(H // P, P).T)
    w1h = W1.astype(np.float16)
    w2h = W2.astype(np.float16)
    ones = np.ones((1, P), dtype=np.float16)
    in_maps = []
    for b in range(B):
        idx = idxs[b]
        K = len(idx)
        Xp = np.zeros((KN, D), dtype=np.float32)
        Xp[:K] = X[b][idx]
        nrm = np.maximum(np.linalg.norm(Xp, axis=1, keepdims=True), 1e-12)
        Xn = Xp / nrm
        xnt = np.ascontiguousarray(
            Xn.T.reshape(2, P, KN).transpose(1, 0, 2)
        ).astype(np.float16)
        in_maps.append(
            {
                "xnt": xnt,
                "x16": Xp.astype(np.float16),
                "w1": w1h,
                "w2": w2h,
                "b1t": b1t,
                "b2t": b2t,
                # 0.5 per padded column (sigmoid(0) contamination) minus the
                # self-loop +1 that is NOT added into sig before the rowsum
                "cvec": np.full(
                    (P, 1), 0.5 * float(KN - K) - 1.0, dtype=np.float32
                ),
                "ones16": ones,
            }
        )
    return in_maps


def kernel(X, mask, W1, b1, W2, b2):
    mask = np.asarray(mask)
    idxs, kb = _plan(mask)
    nc = _get_nc(kb)
    in_maps = make_in_maps(X, mask, W1, b1, W2, b2, idxs, kb)
    res = run_bass_kernel_spmd(nc, in_maps, list(range(B)))
    out = np.zeros((B, N, N), dtype=np.float32)
    for b in range(B):
        idx = idxs[b]
        K = len(idx)
        if K:
            o = np.asarray(res.results[b]["out"])[:K, :K].astype(np.float32)
            out[b][np.ix_(idx, idx)] = o
    return out


# revision 49
# speedup vs baseline: 1.0367x; 1.0367x over previous
"""GCN decoder kernel for Trainium2, 8-core data-parallel over batch.

Key idea: the mask is random 0/1, so only K~=1024 of 2048 nodes are active
per sample, and all masked rows/cols of the output are exactly zero (pair
mask). The computation is permutation-equivariant, so the host packs the
active nodes first (padded to KN = 128*ceil(Kmax/128)) and the device runs
the whole pipeline on [KN, KN] ~= 1/4 of the work. The host scatters the
[K, K] result back into a zero [N, N] matrix.

Host precomputes Xn = X/||X|| (shipped as XnT fp16) so the device does:
  S   = XnT^T XnT;  sig = sigmoid(S) + diag(1)      (f16, ACT)
  deg = rowsum(sig) - 0.5*(KN-K);  d = rsqrt(max(deg, 1e-6))   (DVE,
        rsqrt via bit-hack + 3 Newton steps -- no ACT table switch)
  aggT = d_i * (Y^T @ sig),  Y = d*X (f16)          == (A_norm @ X)^T
  HfT = relu(W1^T aggT + b1);  PT = W2^T HfT + b2   (DVE bias/relu)
  out = sigmoid(PT^T PT)  (f16)  -> host slices [K, K] and scatters.

Padded rows have X=0 -> Xn=0 -> S cols exact 0 -> sigmoid 0.5, corrected in
deg by the host-provided 0.5*(KN-K); padded rows of Y are 0 so they never
contribute; padded output rows/cols are discarded on host.
"""

from contextlib import ExitStack

import ml_dtypes
import numpy as np

import bass_rust as _bass_rust
import concourse.bass as bass
import concourse.mybir as mybir
import concourse.tile as tile
from concourse.bass_utils import run_bass_kernel_spmd
from concourse.masks import make_identity

F32 = mybir.dt.float32
F16 = mybir.dt.float16
F8 = mybir.dt.float8e4
I32 = mybir.dt.int32
AF = mybir.ActivationFunctionType
OP = mybir.AluOpType
DR = mybir.MatmulPerfMode.DoubleRow

B = 8
N = 2048
D = 256
H = 256
P = 128
MAGIC1 = 0x5F3759DF + 1  # fp32 rsqrt bit-hack constant (+1 for the xor trick)


def _install_drain_split(max_waits: int = 1):
    """This walrus build accepts at most ONE sync-wait per instruction.
    (a) split the Tile kernel-tail drain into single-wait drains;
    (b) hoist extra waits from any lowered instruction onto standalone
    EventSemaphore instructions on the same engine."""
    from concourse.vector_clock import ScopedClock

    if getattr(tile.TileContext, "_drain_split_installed", False):
        return

    def _drain_and_barrier(self, tick_clock, wait_clock):
        drain_inst = self.nc.sync.drain()
        wait_clock.add_sem_waits(
            drain_inst.ins, ScopedClock({None: tick_clock.global_clock})
        )
        si = drain_inst.ins.sync_info
        waits = list(si.on_wait) if si is not None and si.on_wait else []
        if len(waits) > max_waits:
            drain_inst.ins.sync_info = _bass_rust.SyncInfo(
                on_wait=waits[:max_waits],
                on_update=list(si.on_update) if si.on_update else [],
            )
            rest = waits[max_waits:]
            for i in range(0, len(rest), max_waits):
                extra = self.nc.sync.drain()
                extra.ins.sync_info = _bass_rust.SyncInfo(
                    on_wait=rest[i : i + max_waits], on_update=[]
                )
        self.nc.all_engine_barrier()
        assert self.sems is not None
        popped = self.nc._tile_sem_poison_stack.pop()
        assert popped is self._sem_poison
        self.nc.clear_and_free_semaphores(list(self.sems.allocated().values()))
        self.nc.all_engine_barrier()

    tile.TileContext._drain_and_barrier = _drain_and_barrier

    orig_add = tile.TileContext._add_instruction
    counter = [0]

    def _add_instruction(self, inst):
        si = inst.sync_info
        if si is not None and si.on_wait and len(si.on_wait) > max_waits:
            waits = list(si.on_wait)
            keep = waits[-max_waits:]
            for w in waits[: -max_waits]:
                counter[0] += 1
                ev = mybir.InstEventSemaphore(
                    name=f"{inst.name}-xw{counter[0]}", ins=[], outs=[]
                )
                ev.engine = inst.engine
                ev.sync_info = _bass_rust.SyncInfo(on_wait=[w], on_update=[])
                orig_add(self, ev)
            inst.sync_info = _bass_rust.SyncInfo(
                on_wait=keep, on_update=list(si.on_update) if si.on_update else []
            )
        orig_add(self, inst)

    tile.TileContext._add_instruction = _add_instruction
    tile.TileContext._drain_split_installed = True


def build_nc(kb: int):
    _install_drain_split()
    KN = P * kb
    KNP = max(KN, 1024)  # psum tile width (>= 1024 for the agg 2-half layout)
    chunks = [(c, min(512, KN - c)) for c in range(0, KN, 512)]

    nc = bass.Bass("TRN2", target_bir_lowering=False, debug=False, num_devices=B)

    xnt_d = nc.dram_tensor("xnt", [P, 2, KN], F8, kind="ExternalInput").ap()
    x16_d = nc.dram_tensor("x16", [KN, D], F16, kind="ExternalInput").ap()
    w1_d = nc.dram_tensor("w1", [D, H], F16, kind="ExternalInput").ap()
    w2_d = nc.dram_tensor("w2", [H, H], F16, kind="ExternalInput").ap()
    b1_d = nc.dram_tensor("b1t", [P, H // P], F32, kind="ExternalInput").ap()
    b2_d = nc.dram_tensor("b2t", [P, H // P], F32, kind="ExternalInput").ap()
    cv_d = nc.dram_tensor("cvec", [P, 1], F32, kind="ExternalInput").ap()
    on_d = nc.dram_tensor("ones16", [1, P], F16, kind="ExternalInput").ap()
    out_d = nc.dram_tensor("out", [KN, KN], F16, kind="ExternalOutput").ap()

    with tile.TileContext(nc) as tc:
        with ExitStack() as top:
            const = top.enter_context(tc.tile_pool(name="const", bufs=1))
            psum = top.enter_context(tc.tile_pool(name="psum", bufs=2, space="PSUM"))
            psd = top.enter_context(tc.tile_pool(name="psd", bufs=1, space="PSUM"))
            outp = top.enter_context(tc.tile_pool(name="outp", bufs=3))

            # ---- input DMAs first (xnt gates phase 2): one tile per
            # (k, 512-col chunk) so the first S matmuls start as soon as
            # the first chunk lands; spread across the three DMA queues ----
            h0 = (KN // 2) // P * P
            xnt = [[None] * len(chunks) for _ in range(2)]
            dma_order = [nc.sync, nc.gpsimd, nc.sync, nc.scalar, nc.scalar, nc.scalar]
            di = 0
            for k in range(2):
                for ci, (c0, cw) in enumerate(chunks):
                    t = const.tile([P, cw], F8, tag=f"xnt{k}_{ci}")
                    dma_order[di % len(dma_order)].dma_start(
                        t[:], xnt_d[:, k, c0 : c0 + cw]
                    )
                    di += 1
                    xnt[k][ci] = t
            x16 = const.tile([P, kb, D], F16, tag="x16")
            w1 = const.tile([P, 2, H], F16, tag="w1")
            w2 = const.tile([P, 2, H], F16, tag="w2")
            b1v = const.tile([P, 2], F32, tag="b1v")
            nc.sync.dma_start(b1v[:], b1_d[:])
            b2v = const.tile([P, 2], F32, tag="b2v")
            nc.sync.dma_start(b2v[:], b2_d[:])
            cvec = const.tile([P, 1], F32, tag="cvec")
            nc.sync.dma_start(cvec[:], cv_d[:])
            ones1 = const.tile([1, P], F16, tag="ones1")
            nc.sync.dma_start(ones1[:], on_d[:])

            # warm up the ACT sigmoid table set right away
            warm = const.tile([1, 2], F32, tag="warm")
            nc.vector.memset(warm[0:1, 0:1], 0.0)
            nc.scalar.activation(warm[0:1, 1:2], warm[0:1, 0:1], AF.Sigmoid)

            eye = const.tile([P, P], F32, tag="eye")
            make_identity(nc, eye[:])

            sig = const.tile([P, kb, KN], F8, tag="sig")
            y16 = const.tile([P, kb, D], F8, tag="y16")
            dly = const.tile([1, 2], F32, tag="dly")
            aggt = const.tile([P, 2, KN], F16, tag="aggt")
            hft = const.tile([P, 2, KN], F16, tag="hft")
            ptt = const.tile([P, 2, KN], F16, tag="ptt")
            dbc = const.tile([P, KN], F16, tag="dbc")
            drow = const.tile([1, KN], F16, tag="drow")
            d16 = const.tile([16, P], F16, tag="d16")
            dgv = const.tile([P, kb], F32, tag="dgv")
            dxh = const.tile([P, kb], F32, tag="dxh")
            dsy = const.tile([P, kb], F32, tag="dsy")
            dnt = const.tile([P, kb], F32, tag="dnt")

            # ---- phase 2: S = Xn Xn^T, sigmoid (f16), diag, rowsum ----
            # deg rowsum: early blocks on the Vector reduce, late blocks via
            # the ACT accumulator (read right after the sigmoid) — balances
            # the two engines; diag +1 is folded into the host cvec.
            NV = min(kb, kb - 3) if kb > 3 else kb
            for jb in range(kb):
                jsl = slice(jb * P, (jb + 1) * P)
                ps = psum.tile([P, KNP], F32, tag="big")
                jc, joff = (jb * P) // 512, (jb * P) % 512
                for k in range(2):
                    for ci, (c0, cw) in enumerate(chunks):
                        nc.tensor.matmul(
                            ps[:, c0 : c0 + cw],
                            xnt[k][jc][:, joff : joff + P],
                            xnt[k][ci][:, 0:cw],
                            start=(k == 0),
                            stop=(k == 1),
                        )
                if jb < NV:
                    nc.scalar.activation(sig[:, jb, :], ps[:, 0:KN], AF.Sigmoid)
                    nc.vector.tensor_reduce(
                        out=dgv[:, jb : jb + 1],
                        in_=sig[:, jb, :],
                        axis=mybir.AxisListType.X,
                        op=OP.add,
                    )
                else:
                    nc.scalar.activation(
                        sig[:, jb, :],
                        ps[:, 0:KN],
                        AF.Sigmoid,
                        accum_out=dgv[:, jb : jb + 1],
                    )
                # add identity on the diagonal block (self loops; padded
                # rows' +1 only touches discarded columns)
                nc.vector.scalar_tensor_tensor(
                    out=sig[:, jb, jsl],
                    in0=eye[:],
                    scalar=1.0,
                    in1=sig[:, jb, jsl],
                    op0=OP.mult,
                    op1=OP.add,
                )

            # x16/w1/w2 are not needed until phases 3/5 — hold their DMA
            # issues behind a data dependency on the first deg slot so their
            # transfers don't compete with xnt's for HBM bandwidth up front
            nc.gpsimd.tensor_tensor(
                dly[0:1, 0:2], dgv[0:1, 0:2], dgv[0:1, 0:2], op=OP.add
            )
            nc.gpsimd.dma_start(x16[:], x16_d.rearrange("(b p) d -> p b d", p=P))
            nc.gpsimd.dma_start(w1[:], w1_d.rearrange("(c p) h -> p c h", p=P))
            nc.gpsimd.dma_start(w2[:], w2_d.rearrange("(c p) h -> p c h", p=P))

            # ---- phase 3: d = rsqrt(max(deg - cvec, 1e-6)), Y, dbc ----
            # Keep-warm matmuls: the PE would otherwise idle > 3.4us across
            # phase 3 and HAM would re-throttle it to 1.2 GHz for phase 4's
            # first chunk. Reading sig[jb] staggers them behind the last
            # sigmoids so the warmth spans the whole gap; unread results.
            for jb in range(max(0, kb - 3), kb):
                for _ in range(3):
                    pwm = psd.tile([P, 1024], F32, tag="psd")
                    nc.tensor.matmul(
                        pwm[:, 0:512],
                        sig[:, jb, 0:P],
                        sig[:, jb, 0:512],
                        start=True,
                        stop=True,
                    )
            nc.vector.tensor_scalar(
                out=dgv[:],
                in0=dgv[:],
                scalar1=cvec[:, 0:1],
                scalar2=1e-6,
                op0=OP.subtract,
                op1=OP.max,
            )
            nc.vector.tensor_scalar_mul(dxh[:], dgv[:], -0.5)  # -x/2
            # y0 bits = MAGIC - (i >> 1)  ==  ((i >> 1) ^ -1) + (MAGIC + 1)
            nc.vector.tensor_scalar(
                out=dsy[:].bitcast(I32),
                in0=dgv[:].bitcast(I32),
                scalar1=1,
                scalar2=-1,
                op0=OP.logical_shift_right,
                op1=OP.bitwise_xor,
            )
            nc.vector.tensor_scalar_add(
                dsy[:].bitcast(I32), dsy[:].bitcast(I32), MAGIC1
            )
            for _ in range(2):  # Newton: y *= 1.5 - 0.5*x*y^2 (err ~5e-6)
                nc.vector.tensor_tensor(dnt[:], dsy[:], dsy[:], op=OP.mult)
                nc.vector.tensor_tensor(dnt[:], dnt[:], dxh[:], op=OP.mult)
                nc.vector.tensor_scalar_add(dnt[:], dnt[:], 1.5)
                nc.vector.tensor_tensor(dsy[:], dsy[:], dnt[:], op=OP.mult)
            # dbc[p, i] = d_i for all p: transpose d, SBUF-SBUF DMA to one
            # row, rank-1 broadcasts. Kick the transpose + DMA chain first;
            # the rank-1 matmuls are emitted after phase 4's first chunk so
            # the PE queue is not head-of-line blocked on the DMA latency.
            ptd = psd.tile([P, 1024], F32, tag="psd")
            nc.tensor.transpose(ptd[0:kb, 0:P], dsy[:], eye[:])
            nc.vector.tensor_copy(out=d16[0:kb, :], in_=ptd[0:kb, 0:P])
            nc.sync.dma_start(
                drow[0:1].rearrange("p (o q) -> p o q", o=kb), d16[0:kb, :]
            )
            # Y = d * X
            for jb in range(kb):
                nc.vector.tensor_scalar_mul(
                    y16[:, jb, :], x16[:, jb, :], dsy[:, jb : jb + 1]
                )

            # ---- phase 4: aggT = dbc * (Y^T @ sig) ----
            # fp8 DoubleRow: contract two jb blocks per matmul
            npair = kb // 2
            for ci, (c0, cw) in enumerate(chunks):
                ps = psum.tile([P, KNP], F32, tag="big")
                for h in range(2):
                    hof = h * P
                    reg = ps[:, 512 * h : 512 * h + cw]
                    for t in range(npair):
                        nc.tensor.matmul(
                            reg,
                            y16[:, 2 * t : 2 * t + 2, hof : hof + P],
                            sig[:, 2 * t : 2 * t + 2, c0 : c0 + cw],
                            start=(t == 0),
                            stop=(t == npair - 1 and kb % 2 == 0),
                            perf_mode=DR,
                        )
                    if kb % 2:
                        nc.tensor.matmul(
                            reg,
                            y16[:, kb - 1, hof : hof + P],
                            sig[:, kb - 1, c0 : c0 + cw],
                            start=(npair == 0),
                            stop=True,
                        )
                if ci == 0:
                    # rank-1 d broadcasts, behind chunk 0's matmuls (the
                    # drow DMA has landed by now); dedicated 2-bank pool so
                    # the main psum rotation is not disturbed
                    for g0 in range(0, KN, 1024):
                        gw = min(1024, KN - g0)
                        pbd = psd.tile([P, 1024], F32, tag="psd")
                        for b0 in range(0, gw, 512):
                            bw = min(512, gw - b0)
                            nc.tensor.matmul(
                                pbd[:, b0 : b0 + bw],
                                ones1[:],
                                drow[0:1, g0 + b0 : g0 + b0 + bw],
                                start=True,
                                stop=True,
                            )
                        nc.vector.tensor_copy(
                            out=dbc[:, g0 : g0 + gw], in_=pbd[:, 0:gw]
                        )
                nc.vector.tensor_tensor(
                    aggt[:, 0, c0 : c0 + cw],
                    ps[:, 0:cw],
                    dbc[:, c0 : c0 + cw],
                    op=OP.mult,
                )
                nc.vector.tensor_tensor(
                    aggt[:, 1, c0 : c0 + cw],
                    ps[:, 512 : 512 + cw],
                    dbc[:, c0 : c0 + cw],
                    op=OP.mult,
                )

            # ---- phase 5: HfT = relu(W1^T aggT + b1), PT = W2^T HfT + b2 ----
            # k-outer with both psum buffers live, so the k=0 matmuls of the
            # second GEMM can run while the first half's drains finish
            pw = [
                psum.tile([P, KNP], F32, tag="big", name=f"pw1_{hb}")
                for hb in range(2)
            ]
            for k in range(2):
                for hb in range(2):
                    hsl = slice(hb * P, (hb + 1) * P)
                    for c0, cw in chunks:
                        nc.tensor.matmul(
                            pw[hb][:, c0 : c0 + cw],
                            w1[:, k, hsl],
                            aggt[:, k, c0 : c0 + cw],
                            start=(k == 0),
                            stop=(k == 1),
                        )
            # hb0 (which gates the next GEMM's k=0 matmuls) drains on ACT,
            # hb1 on Vector, in parallel
            nc.scalar.activation(
                hft[:, 0, :], pw[0][:, 0:KN], AF.Relu, bias=b1v[:, 0:1]
            )
            nc.vector.tensor_scalar(
                out=hft[:, 1, :],
                in0=pw[1][:, 0:KN],
                scalar1=b1v[:, 1:2],
                scalar2=0.0,
                op0=OP.add,
                op1=OP.max,
            )
            pv = [
                psum.tile([P, KNP], F32, tag="big", name=f"pw2_{hb}")
                for hb in range(2)
            ]
            for k in range(2):
                for hb in range(2):
                    hsl = slice(hb * P, (hb + 1) * P)
                    for c0, cw in chunks:
                        nc.tensor.matmul(
                            pv[hb][:, c0 : c0 + cw],
                            w2[:, k, hsl],
                            hft[:, k, c0 : c0 + cw],
                            start=(k == 0),
                            stop=(k == 1),
                        )
            nc.scalar.activation(
                ptt[:, 0, :], pv[0][:, 0:KN], AF.Identity, bias=b2v[:, 0:1]
            )
            nc.vector.tensor_scalar_add(ptt[:, 1, :], pv[1][:, 0:KN], b2v[:, 1:2])

            # ---- phase 6: out = sigmoid(PT^T PT) (f16), DMA out ----
            dma_engs = [nc.sync, nc.gpsimd]
            for jb in range(kb):
                jsl = slice(jb * P, (jb + 1) * P)
                ps = psum.tile([P, KNP], F32, tag="big")
                for k in range(2):
                    for c0, cw in chunks:
                        nc.tensor.matmul(
                            ps[:, c0 : c0 + cw],
                            ptt[:, k, jsl],
                            ptt[:, k, c0 : c0 + cw],
                            start=(k == 0),
                            stop=(k == 1),
                        )
                osb = outp.tile([P, KN], F16, tag="osb")
                nc.scalar.activation(osb[:], ps[:, 0:KN], AF.Sigmoid)
                dma_engs[jb % 2].dma_start(out_d[jsl, :], osb[:])

    return nc


_NC_CACHE: dict[int, object] = {}


def _get_nc(kb: int):
    if kb not in _NC_CACHE:
        _NC_CACHE[kb] = build_nc(kb)
    return _NC_CACHE[kb]


def _plan(mask):
    """Active indices per sample and the shared padded size KN."""
    idxs = [np.nonzero(np.asarray(mask[b]) != 0)[0] for b in range(mask.shape[0])]
    kmax = max((len(i) for i in idxs), default=1)
    kb = max(1, -(-kmax // P))
    return idxs, kb


def make_in_maps(X, mask, W1, b1, W2, b2, idxs, kb):
    KN = P * kb
    X = np.asarray(X, dtype=np.float32)
    W1 = np.asarray(W1, dtype=np.float32)
    b1 = np.asarray(b1, dtype=np.float32)
    W2 = np.asarray(W2, dtype=np.float32)
    b2 = np.asarray(b2, dtype=np.float32)

    b1t = np.ascontiguousarray(b1.reshape(H // P, P).T)
    b2t = np.ascontiguousarray(b2.reshape(H // P, P).T)
    w1h = W1.astype(np.float16)
    w2h = W2.astype(np.float16)
    ones = np.ones((1, P), dtype=np.float16)
    in_maps = []
    for b in range(B):
        idx = idxs[b]
        K = len(idx)
        Xp = np.zeros((KN, D), dtype=np.float32)
        Xp[:K] = X[b][idx]
        nrm = np.maximum(np.linalg.norm(Xp, axis=1, keepdims=True), 1e-12)
        Xn = Xp / nrm
        xnt = np.ascontiguousarray(
            Xn.T.reshape(2, P, KN).transpose(1, 0, 2)
        ).astype(ml_dtypes.float8_e4m3)
        in_maps.append(
            {
                "xnt": xnt,
                "x16": Xp.astype(np.float16),
                "w1": w1h,
                "w2": w2h,
                "b1t": b1t,
                "b2t": b2t,
                # 0.5 per padded column (sigmoid(0) contamination) minus the
                # self-loop +1 that is NOT added into sig before the rowsum
                "cvec": np.full(
                    (P, 1), 0.5 * float(KN - K) - 1.0, dtype=np.float32
                ),
                "ones16": ones,
            }
        )
    return in_maps


def kernel(X, mask, W1, b1, W2, b2):
    mask = np.asarray(mask)
    idxs, kb = _plan(mask)
    nc = _get_nc(kb)
    in_maps = make_in_maps(X, mask, W1, b1, W2, b2, idxs, kb)
    res = run_bass_kernel_spmd(nc, in_maps, list(range(B)))
    out = np.zeros((B, N, N), dtype=np.float32)
    for b in range(B):
        idx = idxs[b]
        K = len(idx)
        if K:
            o = np.asarray(res.results[b]["out"])[:K, :K].astype(np.float32)
            out[b][np.ix_(idx, idx)] = o
    return out
